# revision 14
# baseline (speedup 1.0000x reference)
"""DenseGAT layer on 8 Trainium2 NeuronCores (Bass/Tile).

Math (reference):
  h = (x @ W.T).reshape(N, H, F)
  src[n,h] = <h[n,h,:], attn_src[h,:]> ; dst likewise
  logits[i,j,h] = leaky_relu(src[i,h] + dst[j,h], 0.2)
  logits = where(adj[i,j], logits, -1e4); alpha = softmax over j
  out[i,f] = mean_h sum_j alpha[i,j,h] h[j,h,f]

Kernel identities used:
  leaky_relu(s, 0.2) == max(s, 0.2*s)  (one fused DVE scalar_tensor_tensor)
  masked entries contribute exactly 0 (multiply by 0/1 mask after exp)
  softmax without max-subtraction is exact here (|s| <= ~7, clamped at 11)

Sharding: 1D row partition of destination nodes i across 8 cores (512 rows
each).  Each core gets full x^T (replicated) and its own slice of
adjacency^T (j-major [4096, 512]).  Softmax over j is local to a core.

Per core:
  pass 1: h[j,:] via PE in bf16 (lhsT = x^T chunks, rhs = W^T); src/dst
          scores via PE in fp32 with a host-folded [256,16] projection
          bmat = W.T @ blockdiag(attn) (scores = x @ bmat).  src scores
          roundtrip through DRAM so each core selects its own 512-row
          shard with a partition_id()-dynamic DMA offset.
  pass 2: per j-tile t (128 source nodes):
          s[j,(h,i)] = src_bcast + dst[j,h]  fp16   (DVE TS, clamp 11, 4x)
          l = max(s, 0.2*s)                  fp16   (DVE stt leaky)
          e = Exp(l)                         fp16   (ACT, one wide sweep)
          P = e * maskT[j,:]                 fp16   (DVE TT, mask bcast)
          psum_o[h][0:65, i] += [h_h | 1].T @ P[:, h, :]   (PE, one
            accumulation group per PSUM bank; out row 64 = denominator)
  epilogue: rinv = (1/H)/den (DVE reciprocal), partition-broadcast of each
          rinv row via DRAM roundtrip DMA, out^T[f,i] = sum_h num_h * rinv_h
          (DVE), DMA out.  Host transposes the [F, S] shard.

Engine assignment: ACT does the exp + PSUM drains; DVE does s/leaky/mask +
epilogue; Pool (gpsimd) does mask u8->f16 converts, x^T bf16 casts and
h_aug assembly; PE does all matmuls; big DMAs ride HWDGE (nc.sync).

Platform constraints honored: every instruction carries at most ONE
semaphore wait (engines "touch" fresh DMA tiles first; multi-DMA producers
are funneled into single DMAs), and a PSUM bank hosts only one matmul
accumulation group at a time (interleaved start/stop groups within a bank
corrupt accumulation -- verified on HW).
"""
import numpy as np

import concourse.mybir as mybir
from concourse.bass import AP
from concourse.bacc import Bacc
from concourse.tile import TileContext
from concourse import bass_utils

N = 4096          # nodes
D = 256           # input dim
H = 8             # heads
F = 64            # hidden per head
HF = H * F        # 512
NC = 8            # cores
S = N // NC       # 512 destination rows per core
T = N // 128      # 32 source-node tiles
CLAMP = 11.0      # exp(11) < fp16 max; real logits are ~|s|<=7

f32 = mybir.dt.float32
f16 = mybir.dt.float16
bf16 = mybir.dt.bfloat16
u8 = mybir.dt.uint8

_CACHE = {}


def build(debug=False, t_limit=None):
    nc = Bacc("TRN2", target_bir_lowering=False)

    xT = nc.dram_tensor("xT", [D, N], f32, kind="ExternalInput")
    WT = nc.dram_tensor("WT", [D, HF], bf16, kind="ExternalInput")
    bmat = nc.dram_tensor("bmat", [D, 16], f32, kind="ExternalInput")
    maskT = nc.dram_tensor("maskT", [N, S], u8, kind="ExternalInput")
    out = nc.dram_tensor("out", [F, S], f32, kind="ExternalOutput")
    if debug:
        dbg_scores = nc.dram_tensor("dbg_scores", [128, T, 16], f32, kind="ExternalOutput")
        dbg_srcb = nc.dram_tensor("dbg_srcb", [128, H, S], f16, kind="ExternalOutput")
        dbg_pm = nc.dram_tensor("dbg_pm", [128, H, S], f16, kind="ExternalOutput")
        dbg_ps0 = nc.dram_tensor("dbg_ps0", [65, S], f32, kind="ExternalOutput")
        dbg_ps1 = nc.dram_tensor("dbg_ps1", [65, S], f32, kind="ExternalOutput")

    T2 = T if t_limit is None else t_limit

    with TileContext(nc) as tc:
        with tc.tile_pool(name="pers", bufs=1) as pers, \
             tc.tile_pool(name="dram", bufs=1, space="DRAM") as dpool:
            h_all = pers.tile([128, T, HF], f32)         # 64KB/part
            scores_all = pers.tile([128, T, 16], f32)    # 2KB/part
            src_b = pers.tile([128, H, S], f16)          # 8KB/part
            scr = dpool.tile([N, H], f32)                # DRAM scratch, j-major
            scr2 = dpool.tile([H, S], f32)               # DRAM scratch for rinv

            # ---------------- pass 1: h and scores ----------------
            with tc.tile_pool(name="p1", bufs=1) as p1, \
                 tc.tile_pool(name="ps1", bufs=2, space="PSUM") as ps1:
                xT_sb = p1.tile([128, 2, N], f32)        # 32KB/part
                xT_bf = p1.tile([128, 2, N], bf16)       # 16KB/part
                WT_sb = p1.tile([128, 2, HF], bf16)
                bm_sb = p1.tile([128, 2, 16], f32)
                src_b_ih = p1.tile([128, S, H], f16)     # 8KB/part, transient
                for kh in range(2):
                    nc.sync.dma_start(out=xT_sb[:, kh, :], in_=xT[kh * 128:(kh + 1) * 128, :])
                    nc.sync.dma_start(out=WT_sb[:, kh, :], in_=WT[kh * 128:(kh + 1) * 128, :])
                    nc.sync.dma_start(out=bm_sb[:, kh, :], in_=bmat[kh * 128:(kh + 1) * 128, :])

                # PE touches: observe each fresh queue sem once (1-wait limit)
                scrp = ps1.tile([128, 8], f32, tag="scrp")
                for ap in (xT_sb[:, 0, 0:1], xT_sb[:, 1, 0:1],
                           bm_sb[:, 0, 0:1], bm_sb[:, 1, 0:1]):
                    nc.tensor.matmul(scrp[0:1, 0:1], ap, ap, start=True, stop=True)
                nc.tensor.ldweights(WT_sb[:, 0, 0:1])
                nc.tensor.ldweights(WT_sb[:, 1, 0:1])

                # scores FIRST (fast) so the src roundtrip unlocks DVE early
                for t in range(T):
                    psc = ps1.tile([128, 16], f32, tag="psc")
                    for kh in range(2):
                        nc.tensor.matmul(psc, xT_sb[:, kh, t * 128:(t + 1) * 128],
                                         bm_sb[:, kh, :],
                                         start=(kh == 0), stop=(kh == 1))
                    nc.scalar.copy(scores_all[:, t, :], psc)

                # src scores -> DRAM scratch (j-major), one DMA
                # element (p, t, h) -> scr[(t*128+p)*H + h]
                scr_w = AP(tensor=scr.tensor, offset=scr.offset,
                           ap=[[H, 128], [128 * H, T], [1, H]])
                nc.sync.dma_start(out=scr_w, in_=scores_all[:, :, 0:H])

                # read back own shard, broadcast over partitions: [128, S, H]
                # (dynamic offset -> SWDGE/gpsimd)
                pid = nc.gpsimd.partition_id()
                scr_r = AP(tensor=scr.tensor, offset=scr.offset + pid * (S * H),
                           ap=[[0, 128], [H, S], [1, H]])
                nc.gpsimd.dma_start(out=src_b_ih, in_=scr_r)
                # transpose (i,h) -> (h,i) once so pass-2 reads are contiguous
                src_view = AP(tensor=src_b_ih.tensor, offset=src_b_ih.offset,
                              ap=[src_b_ih.ap[0], [1, H], [H, S]])
                nc.vector.tensor_copy(src_b, src_view)

                # Pool casts x^T to bf16 (its first read of each xT DMA queue)
                for kh in range(2):
                    nc.gpsimd.tensor_copy(xT_bf[:, kh, :], xT_sb[:, kh, :])
                nc.tensor.ldweights(xT_bf[:, 0, 0:1])   # Pool sem touch

                for t in range(T):
                    ph = ps1.tile([128, HF], f32, tag="ph")
                    for kh in range(2):
                        nc.tensor.matmul(ph, xT_bf[:, kh, t * 128:(t + 1) * 128],
                                         WT_sb[:, kh, :],
                                         start=(kh == 0), stop=(kh == 1))
                    nc.scalar.copy(h_all[:, t, :], ph)

            if debug:
                nc.sync.dma_start(out=dbg_scores[:, :, :], in_=scores_all)
                nc.sync.dma_start(out=dbg_srcb[:, :, :], in_=src_b)

            # ---------------- pass 2: attention + aggregation ----------------
            with tc.tile_pool(name="pm", bufs=3) as pm, \
                 tc.tile_pool(name="pmf", bufs=3) as pmf, \
                 tc.tile_pool(name="psl", bufs=2) as psl, \
                 tc.tile_pool(name="pll", bufs=2) as pll, \
                 tc.tile_pool(name="pee", bufs=3) as pee, \
                 tc.tile_pool(name="ppm", bufs=3) as ppm, \
                 tc.tile_pool(name="pha", bufs=3) as pha, \
                 tc.tile_pool(name="pep", bufs=1) as pep:

                oh_sb = pep.tile([128, H, S], f32)   # 16KB/part (rows 0..64)
                dve_scr = pep.tile([1, 1], f32, tag="dve_scr", name="dve_scr")

                with tc.tile_pool(name="pso", bufs=1, space="PSUM") as pso:
                    psum_o = [pso.tile([128, S], f32, tag=f"o{h}", name=f"psum_o{h}")
                              for h in range(H)]

                    for t in range(T2):
                        m_sb = pm.tile([128, S], u8)
                        nc.sync.dma_start(out=m_sb, in_=maskT[t * 128:(t + 1) * 128, :])
                        m_f = pmf.tile([128, S], f16)
                        nc.gpsimd.tensor_copy(m_f, m_sb)   # also the Pool "touch"

                        s_all = psl.tile([128, H, S], f16)
                        for h in range(H):
                            nc.vector.tensor_scalar(
                                out=s_all[:, h, :], in0=src_b[:, h, :],
                                scalar1=scores_all[:, t, 8 + h:9 + h], scalar2=CLAMP,
                                op0=mybir.AluOpType.add, op1=mybir.AluOpType.min)
                        # leaky: l = max(0.2*s, s); TS(4x) + TT(2x) beats
                        # the fused stt (1x only) on DVE; max lands in-place
                        l_all = pll.tile([128, H, S], f16, tag="l_all", name="l_all")
                        nc.vector.tensor_scalar_mul(l_all, s_all, 0.2)
                        nc.vector.tensor_tensor(out=l_all, in0=s_all, in1=l_all,
                                                op=mybir.AluOpType.max)

                        e_all = pee.tile([128, H, S], f16)
                        nc.scalar.activation(out=e_all, in_=l_all,
                                             func=mybir.ActivationFunctionType.Exp,
                                             bias=0.0, scale=1.0)

                        # DVE touch of m_f's Pool sem before the 2-dep TT
                        nc.vector.tensor_copy(dve_scr, m_f[0:1, 0:1])
                        Pm = ppm.tile([128, H, S], f16)
                        m_bc = AP(tensor=m_f.tensor, offset=m_f.offset,
                                  ap=[m_f.ap[0], [0, H], [1, S]])
                        nc.vector.tensor_tensor(out=Pm, in0=e_all, in1=m_bc,
                                                op=mybir.AluOpType.mult)

                        # h_aug on Pool: [h | 1], fp16
                        ha = pha.tile([128, H, 65], f16)
                        nc.gpsimd.memset(ha[:, :, 64:65], 1.0)
                        h_view = AP(tensor=h_all.tensor,
                                    offset=h_all.offset + t * HF,
                                    ap=[h_all.ap[0], [F, H], [1, F]])
                        nc.gpsimd.tensor_copy(ha[:, :, 0:64], h_view)

                        if debug and t == 0:
                            nc.sync.dma_start(out=dbg_pm[:, :, :], in_=Pm)

                        # PE touch of ha (Pool sem), then matmuls wait on DVE only
                        nc.tensor.ldweights(ha[:, 0, 0:1])
                        # one accumulation group per PSUM bank: out^T layout
                        for h in range(H):
                            nc.tensor.matmul(
                                psum_o[h][0:65, :],
                                ha[:, h, :],
                                Pm[:, h, :],
                                start=(t == 0), stop=(t == T2 - 1))

                    # drain PSUM accumulators to SBUF (inside pso scope)
                    for h in range(H):
                        nc.scalar.copy(oh_sb[0:65, h, :], psum_o[h][0:65, :])

                # ---- epilogue (transposed layout [f, i]) ----
                if debug:
                    nc.sync.dma_start(out=dbg_ps0[:, :], in_=oh_sb[0:65, 0, :])
                    nc.sync.dma_start(out=dbg_ps1[:, :], in_=oh_sb[0:65, 1, :])

                # den rows (partition 64, col h) -> den_all partitions 0..7, one DMA
                den_all = pep.tile([H, S], f32)
                nc.sync.dma_start(out=den_all[:, :], in_=oh_sb[64:65, :, :])
                rinv = pep.tile([H, S], f32)
                nc.vector.reciprocal(rinv, den_all)
                nc.vector.tensor_scalar_mul(rinv, rinv, 1.0 / H)
                # roundtrip rinv through DRAM for partition-broadcast reads
                nc.sync.dma_start(out=scr2[:, :], in_=rinv)

                accA = pep.tile([F, S], f32, tag="accA", name="accA")
                accB = pep.tile([F, S], f32, tag="accB", name="accB")
                rb_sb = pep.tile([F, H, S], f32, tag="rb_sb", name="rb_sb")
                # DVE touch of oh_sb's ACT sem before combining with DMA'd rb
                nc.vector.tensor_copy(dve_scr, oh_sb[0:1, 0, 0:1])

                for h in range(H):   # prefetch all broadcasts up front
                    rb_src = AP(tensor=scr2.tensor, offset=scr2.offset + h * S,
                                ap=[[0, F], [1, S]])
                    nc.sync.dma_start(out=rb_sb[:, h, :], in_=rb_src)
                for h in range(H):
                    rb = rb_sb[:, h, :]
                    src_n = oh_sb[0:F, h, :]
                    if h == 0:
                        nc.vector.tensor_tensor(out=accA, in0=src_n, in1=rb,
                                                op=mybir.AluOpType.mult)
                    else:
                        # rb <- num*rinv (in place), then acc += rb
                        nc.vector.tensor_tensor(out=rb, in0=src_n, in1=rb,
                                                op=mybir.AluOpType.mult)
                        dst_acc = accB if h % 2 == 1 else accA
                        src_acc = accA if h % 2 == 1 else accB
                        nc.vector.tensor_tensor(out=dst_acc, in0=src_acc,
                                                in1=rb, op=mybir.AluOpType.add)
                nc.sync.dma_start(out=out[:, :], in_=accB)  # h=7 -> accB

    nc.compile()
    return nc


def _get_nc():
    if "nc" not in _CACHE:
        _CACHE["nc"] = build()
    return _CACHE["nc"]


def make_in_maps(x, adjacency, W, attn_src, attn_dst):
    import ml_dtypes
    x = np.asarray(x, dtype=np.float32)
    W = np.asarray(W, dtype=np.float32)
    attn_src = np.asarray(attn_src, dtype=np.float32)
    attn_dst = np.asarray(attn_dst, dtype=np.float32)
    adj = np.asarray(adjacency)

    xT = np.ascontiguousarray(x.T)                        # [D, N] f32
    WT = np.ascontiguousarray(W.T).astype(ml_dtypes.bfloat16)  # [D, HF] bf16
    W3 = W.reshape(H, F, D)
    bsrc = np.einsum("hfd,hf->dh", W3, attn_src)          # [D, H]
    bdst = np.einsum("hfd,hf->dh", W3, attn_dst)          # [D, H]
    bmat = np.ascontiguousarray(
        np.concatenate([bsrc, bdst], axis=1).astype(np.float32))  # [D, 16]
    maskT_full = np.ascontiguousarray(adj.T).view(np.uint8)        # [N, N]

    return [dict(xT=xT, WT=WT, bmat=bmat,
                 maskT=np.ascontiguousarray(maskT_full[:, c * S:(c + 1) * S]))
            for c in range(NC)]


def kernel(x, adjacency, W, attn_src, attn_dst):
    in_maps = make_in_maps(x, adjacency, W, attn_src, attn_dst)
    nc = _get_nc()
    res = bass_utils.run_bass_kernel_spmd(nc, in_maps, core_ids=list(range(NC)))
    # each core returns out^T [F, S]; transpose and stack shards
    return np.concatenate([r["out"].T for r in res.results], axis=0)


# revision 18
# speedup vs baseline: 1.0813x; 1.0813x over previous
"""DenseGAT layer on 8 Trainium2 NeuronCores (Bass/Tile).

Math (reference):
  h = (x @ W.T).reshape(N, H, F)
  src[n,h] = <h[n,h,:], attn_src[h,:]> ; dst likewise
  logits[i,j,h] = leaky_relu(src[i,h] + dst[j,h], 0.2)
  logits = where(adj[i,j], logits, -1e4); alpha = softmax over j
  out[i,f] = mean_h sum_j alpha[i,j,h] h[j,h,f]

Kernel identities used:
  leaky_relu(s, 0.2) == max(s, 0.2*s)  (one fused DVE scalar_tensor_tensor)
  masked entries contribute exactly 0 (multiply by 0/1 mask after exp)
  softmax without max-subtraction is exact here (|s| <= ~7, clamped at 11)

Sharding: 1D row partition of destination nodes i across 8 cores (512 rows
each).  Each core gets full x^T (replicated) and its own slice of
adjacency^T (j-major [4096, 512]).  Softmax over j is local to a core.

Per core:
  pass 1: h[j,:] via PE in bf16 (lhsT = x^T chunks, rhs = W^T); src/dst
          scores via PE in fp32 with a host-folded [256,16] projection
          bmat = W.T @ blockdiag(attn) (scores = x @ bmat).  src scores
          roundtrip through DRAM so each core selects its own 512-row
          shard with a partition_id()-dynamic DMA offset.
  pass 2: per j-tile t (128 source nodes):
          s[j,(h,i)] = src_bcast + dst[j,h]  fp16   (DVE TS, clamp 11, 4x)
          l = max(s, 0.2*s)                  fp16   (DVE stt leaky)
          e = Exp(l)                         fp16   (ACT, one wide sweep)
          P = e * maskT[j,:]                 fp16   (DVE TT, mask bcast)
          psum_o[h][0:65, i] += [h_h | 1].T @ P[:, h, :]   (PE, one
            accumulation group per PSUM bank; out row 64 = denominator)
  epilogue: rinv = (1/H)/den (DVE reciprocal), partition-broadcast of each
          rinv row via DRAM roundtrip DMA, out^T[f,i] = sum_h num_h * rinv_h
          (DVE), DMA out.  Host transposes the [F, S] shard.

Engine assignment: ACT does the exp + PSUM drains; DVE does s/leaky/mask +
epilogue; Pool (gpsimd) does mask u8->f16 converts, x^T bf16 casts and
h_aug assembly; PE does all matmuls; big DMAs ride HWDGE (nc.sync).

Platform constraints honored: every instruction carries at most ONE
semaphore wait (engines "touch" fresh DMA tiles first; multi-DMA producers
are funneled into single DMAs), and a PSUM bank hosts only one matmul
accumulation group at a time (interleaved start/stop groups within a bank
corrupt accumulation -- verified on HW).
"""
import numpy as np

import concourse.mybir as mybir
from concourse.bass import AP
from concourse.bacc import Bacc
from concourse.tile import TileContext
from concourse import bass_utils

N = 4096          # nodes
D = 256           # input dim
H = 8             # heads
F = 64            # hidden per head
HF = H * F        # 512
NC = 8            # cores
S = N // NC       # 512 destination rows per core
T = N // 128      # 32 source-node tiles
CLAMP = 11.0      # exp(11) < fp16 max; real logits are ~|s|<=7

f32 = mybir.dt.float32
f16 = mybir.dt.float16
bf16 = mybir.dt.bfloat16
u8 = mybir.dt.uint8

_CACHE = {}


def build(debug=False, t_limit=None):
    nc = Bacc("TRN2", target_bir_lowering=False)

    xT = nc.dram_tensor("xT", [D, N], f32, kind="ExternalInput")
    WT = nc.dram_tensor("WT", [D, HF], bf16, kind="ExternalInput")
    bmat = nc.dram_tensor("bmat", [D, 16], f32, kind="ExternalInput")
    maskT = nc.dram_tensor("maskT", [N, S], u8, kind="ExternalInput")
    out = nc.dram_tensor("out", [F, S], f32, kind="ExternalOutput")
    if debug:
        dbg_scores = nc.dram_tensor("dbg_scores", [128, T, 16], f32, kind="ExternalOutput")
        dbg_srcb = nc.dram_tensor("dbg_srcb", [128, H, S], f16, kind="ExternalOutput")
        dbg_pm = nc.dram_tensor("dbg_pm", [128, H, S], f16, kind="ExternalOutput")
        dbg_ps0 = nc.dram_tensor("dbg_ps0", [65, S], f32, kind="ExternalOutput")
        dbg_ps1 = nc.dram_tensor("dbg_ps1", [65, S], f32, kind="ExternalOutput")

    T2 = T if t_limit is None else t_limit

    with TileContext(nc) as tc:
        with tc.tile_pool(name="pers", bufs=1) as pers, \
             tc.tile_pool(name="dram", bufs=1, space="DRAM") as dpool:
            h_all = pers.tile([128, T, HF], f32)         # 64KB/part
            scores_all = pers.tile([128, T, 16], f32)    # 2KB/part
            src_b = pers.tile([128, H, S], f16)          # 8KB/part
            scr = dpool.tile([N, H], f32)                # DRAM scratch, j-major
            scr2 = dpool.tile([H, S], f32)               # DRAM scratch for rinv

            # ---------------- pass 1: h and scores ----------------
            with tc.tile_pool(name="p1", bufs=1) as p1, \
                 tc.tile_pool(name="ps1", bufs=2, space="PSUM") as ps1:
                xT_sb = p1.tile([128, 2, N], f32)        # 32KB/part
                xT_bf = p1.tile([128, 2, N], bf16)       # 16KB/part
                WT_sb = p1.tile([128, 2, HF], bf16)
                bm_sb = p1.tile([128, 2, 16], f32)
                src_b_ih = p1.tile([128, S, H], f16)     # 8KB/part, transient
                NCHUNK = 4          # xT arrives in column chunks so the
                CW = N // NCHUNK    # score matmuls can start early
                for kh in range(2):
                    nc.sync.dma_start(out=bm_sb[:, kh, :], in_=bmat[kh * 128:(kh + 1) * 128, :])
                    nc.sync.dma_start(out=WT_sb[:, kh, :], in_=WT[kh * 128:(kh + 1) * 128, :])
                for cchunk in range(NCHUNK):
                    for kh in range(2):
                        nc.sync.dma_start(
                            out=xT_sb[:, kh, cchunk * CW:(cchunk + 1) * CW],
                            in_=xT[kh * 128:(kh + 1) * 128, cchunk * CW:(cchunk + 1) * CW])

                # PE touches: observe each fresh queue sem once (1-wait limit)
                scrp = ps1.tile([128, 8], f32, tag="scrp")
                for ap in (bm_sb[:, 0, 0:1], bm_sb[:, 1, 0:1]):
                    nc.tensor.matmul(scrp[0:1, 0:1], ap, ap, start=True, stop=True)
                nc.tensor.ldweights(WT_sb[:, 0, 0:1])
                nc.tensor.ldweights(WT_sb[:, 1, 0:1])
                for cchunk in range(NCHUNK):
                    for kh in range(2):
                        ap = xT_sb[:, kh, cchunk * CW:cchunk * CW + 1]
                        nc.tensor.matmul(scrp[0:1, 0:1], ap, ap, start=True, stop=True)

                # scores FIRST (fast) so the src roundtrip unlocks DVE early
                for t in range(T):
                    psc = ps1.tile([128, 16], f32, tag="psc")
                    for kh in range(2):
                        nc.tensor.matmul(psc, xT_sb[:, kh, t * 128:(t + 1) * 128],
                                         bm_sb[:, kh, :],
                                         start=(kh == 0), stop=(kh == 1))
                    nc.scalar.copy(scores_all[:, t, :], psc)

                # src scores -> DRAM scratch (j-major), one DMA
                # element (p, t, h) -> scr[(t*128+p)*H + h]
                scr_w = AP(tensor=scr.tensor, offset=scr.offset,
                           ap=[[H, 128], [128 * H, T], [1, H]])
                nc.sync.dma_start(out=scr_w, in_=scores_all[:, :, 0:H])

                # read back own shard, broadcast over partitions: [128, S, H]
                # (dynamic offset -> SWDGE/gpsimd)
                pid = nc.gpsimd.partition_id()
                scr_r = AP(tensor=scr.tensor, offset=scr.offset + pid * (S * H),
                           ap=[[0, 128], [H, S], [1, H]])
                nc.gpsimd.dma_start(out=src_b_ih, in_=scr_r)
                # transpose (i,h) -> (h,i) once so pass-2 reads are contiguous
                src_view = AP(tensor=src_b_ih.tensor, offset=src_b_ih.offset,
                              ap=[src_b_ih.ap[0], [1, H], [H, S]])
                nc.vector.tensor_copy(src_b, src_view)

                # Pool casts x^T to bf16, per chunk (one queue sem each)
                for cchunk in range(NCHUNK):
                    for kh in range(2):
                        nc.gpsimd.tensor_copy(
                            xT_bf[:, kh, cchunk * CW:(cchunk + 1) * CW],
                            xT_sb[:, kh, cchunk * CW:(cchunk + 1) * CW])
                nc.tensor.ldweights(xT_bf[:, 0, 0:1])   # Pool sem touch

                for t in range(T):
                    ph = ps1.tile([128, HF], f32, tag="ph")
                    for kh in range(2):
                        nc.tensor.matmul(ph, xT_bf[:, kh, t * 128:(t + 1) * 128],
                                         WT_sb[:, kh, :],
                                         start=(kh == 0), stop=(kh == 1))
                    nc.scalar.copy(h_all[:, t, :], ph)

            if debug:
                nc.sync.dma_start(out=dbg_scores[:, :, :], in_=scores_all)
                nc.sync.dma_start(out=dbg_srcb[:, :, :], in_=src_b)

            # ---------------- pass 2: attention + aggregation ----------------
            with tc.tile_pool(name="pm", bufs=3) as pm, \
                 tc.tile_pool(name="pmf", bufs=3) as pmf, \
                 tc.tile_pool(name="psl", bufs=2) as psl, \
                 tc.tile_pool(name="pll", bufs=2) as pll, \
                 tc.tile_pool(name="pee", bufs=2) as pee, \
                 tc.tile_pool(name="ppm", bufs=3) as ppm, \
                 tc.tile_pool(name="pha", bufs=3) as pha, \
                 tc.tile_pool(name="pep", bufs=1) as pep:

                oh_sb = pep.tile([128, H, S], f32)   # 16KB/part (rows 0..64)
                dve_scr = pep.tile([1, 1], f32, tag="dve_scr", name="dve_scr")

                with tc.tile_pool(name="pso", bufs=1, space="PSUM") as pso:
                    psum_o = [pso.tile([128, S], f32, tag=f"o{h}", name=f"psum_o{h}")
                              for h in range(H)]

                    for t in range(T2):
                        m_sb = pm.tile([128, S], u8)
                        nc.sync.dma_start(out=m_sb, in_=maskT[t * 128:(t + 1) * 128, :])
                        m_f = pmf.tile([128, S], f16)
                        nc.gpsimd.tensor_copy(m_f, m_sb)   # also the Pool "touch"

                        s_all = psl.tile([128, H, S], f16)
                        for h in range(H):
                            nc.vector.tensor_scalar(
                                out=s_all[:, h, :], in0=src_b[:, h, :],
                                scalar1=scores_all[:, t, 8 + h:9 + h], scalar2=CLAMP,
                                op0=mybir.AluOpType.add, op1=mybir.AluOpType.min)
                        # Head-split hybrid to balance DVE vs ACT:
                        #  heads 0..3: l = max(s, 0.2s) on DVE, one Exp on ACT
                        #  heads 4..7: e = max(Exp(s), Exp(0.2s)): 2 Exps on
                        #  ACT, max on DVE
                        HL = H // 2
                        e_all = pee.tile([128, H, S], f16)
                        l_lo = pll.tile([128, HL, S], f16, tag="l_lo", name="l_lo")
                        nc.vector.tensor_scalar_mul(l_lo, s_all[:, 0:HL, :], 0.2)
                        nc.vector.tensor_tensor(out=l_lo, in0=s_all[:, 0:HL, :],
                                                in1=l_lo, op=mybir.AluOpType.max)
                        nc.scalar.activation(out=e_all[:, 0:HL, :], in_=l_lo,
                                             func=mybir.ActivationFunctionType.Exp,
                                             bias=0.0, scale=1.0)
                        e1h = pll.tile([128, HL, S], f16, tag="e1h", name="e1h")
                        nc.scalar.activation(out=e1h, in_=s_all[:, HL:H, :],
                                             func=mybir.ActivationFunctionType.Exp,
                                             bias=0.0, scale=1.0)
                        e2h = pll.tile([128, HL, S], f16, tag="e2h", name="e2h")
                        nc.scalar.activation(out=e2h, in_=s_all[:, HL:H, :],
                                             func=mybir.ActivationFunctionType.Exp,
                                             bias=0.0, scale=0.2)
                        nc.vector.tensor_tensor(out=e_all[:, HL:H, :], in0=e1h,
                                                in1=e2h, op=mybir.AluOpType.max)

                        # DVE touch of m_f's Pool sem before the 2-dep TT
                        nc.vector.tensor_copy(dve_scr, m_f[0:1, 0:1])
                        Pm = ppm.tile([128, H, S], f16)
                        m_bc = AP(tensor=m_f.tensor, offset=m_f.offset,
                                  ap=[m_f.ap[0], [0, H], [1, S]])
                        nc.vector.tensor_tensor(out=Pm, in0=e_all, in1=m_bc,
                                                op=mybir.AluOpType.mult)

                        # h_aug on Pool: [h | 1], fp16
                        ha = pha.tile([128, H, 65], f16)
                        nc.gpsimd.memset(ha[:, :, 64:65], 1.0)
                        h_view = AP(tensor=h_all.tensor,
                                    offset=h_all.offset + t * HF,
                                    ap=[h_all.ap[0], [F, H], [1, F]])
                        nc.gpsimd.tensor_copy(ha[:, :, 0:64], h_view)

                        if debug and t == 0:
                            nc.sync.dma_start(out=dbg_pm[:, :, :], in_=Pm)

                        # PE touch of ha (Pool sem), then matmuls wait on DVE only
                        nc.tensor.ldweights(ha[:, 0, 0:1])
                        # one accumulation group per PSUM bank: out^T layout
                        for h in range(H):
                            nc.tensor.matmul(
                                psum_o[h][0:65, :],
                                ha[:, h, :],
                                Pm[:, h, :],
                                start=(t == 0), stop=(t == T2 - 1))

                    # drain PSUM accumulators to SBUF (inside pso scope)
                    for h in range(H):
                        nc.scalar.copy(oh_sb[0:65, h, :], psum_o[h][0:65, :])

                # ---- epilogue (transposed layout [f, i]) ----
                if debug:
                    nc.sync.dma_start(out=dbg_ps0[:, :], in_=oh_sb[0:65, 0, :])
                    nc.sync.dma_start(out=dbg_ps1[:, :], in_=oh_sb[0:65, 1, :])

                # den rows (partition 64, col h) -> den_all partitions 0..7, one DMA
                den_all = pep.tile([H, S], f32)
                nc.sync.dma_start(out=den_all[:, :], in_=oh_sb[64:65, :, :])
                rinv = pep.tile([H, S], f32)
                nc.vector.reciprocal(rinv, den_all)
                nc.vector.tensor_scalar_mul(rinv, rinv, 1.0 / H)
                # roundtrip rinv through DRAM for partition-broadcast reads
                nc.sync.dma_start(out=scr2[:, :], in_=rinv)

                accA = pep.tile([F, S], f32, tag="accA", name="accA")
                accB = pep.tile([F, S], f32, tag="accB", name="accB")
                rb_sb = pep.tile([F, H, S], f32, tag="rb_sb", name="rb_sb")
                # DVE touch of oh_sb's ACT sem before combining with DMA'd rb
                nc.vector.tensor_copy(dve_scr, oh_sb[0:1, 0, 0:1])

                for h in range(H):   # prefetch all broadcasts up front
                    rb_src = AP(tensor=scr2.tensor, offset=scr2.offset + h * S,
                                ap=[[0, F], [1, S]])
                    nc.sync.dma_start(out=rb_sb[:, h, :], in_=rb_src)
                for h in range(H):
                    rb = rb_sb[:, h, :]
                    src_n = oh_sb[0:F, h, :]
                    if h == 0:
                        nc.vector.tensor_tensor(out=accA, in0=src_n, in1=rb,
                                                op=mybir.AluOpType.mult)
                    else:
                        # rb <- num*rinv (in place), then acc += rb
                        nc.vector.tensor_tensor(out=rb, in0=src_n, in1=rb,
                                                op=mybir.AluOpType.mult)
                        dst_acc = accB if h % 2 == 1 else accA
                        src_acc = accA if h % 2 == 1 else accB
                        nc.vector.tensor_tensor(out=dst_acc, in0=src_acc,
                                                in1=rb, op=mybir.AluOpType.add)
                nc.sync.dma_start(out=out[:, :], in_=accB)  # h=7 -> accB

    nc.compile()
    return nc


def _get_nc():
    if "nc" not in _CACHE:
        _CACHE["nc"] = build()
    return _CACHE["nc"]


def make_in_maps(x, adjacency, W, attn_src, attn_dst):
    import ml_dtypes
    x = np.asarray(x, dtype=np.float32)
    W = np.asarray(W, dtype=np.float32)
    attn_src = np.asarray(attn_src, dtype=np.float32)
    attn_dst = np.asarray(attn_dst, dtype=np.float32)
    adj = np.asarray(adjacency)

    xT = np.ascontiguousarray(x.T)                        # [D, N] f32
    WT = np.ascontiguousarray(W.T).astype(ml_dtypes.bfloat16)  # [D, HF] bf16
    W3 = W.reshape(H, F, D)
    bsrc = np.einsum("hfd,hf->dh", W3, attn_src)          # [D, H]
    bdst = np.einsum("hfd,hf->dh", W3, attn_dst)          # [D, H]
    bmat = np.ascontiguousarray(
        np.concatenate([bsrc, bdst], axis=1).astype(np.float32))  # [D, 16]
    maskT_full = np.ascontiguousarray(adj.T).view(np.uint8)        # [N, N]

    return [dict(xT=xT, WT=WT, bmat=bmat,
                 maskT=np.ascontiguousarray(maskT_full[:, c * S:(c + 1) * S]))
            for c in range(NC)]


def kernel(x, adjacency, W, attn_src, attn_dst):
    in_maps = make_in_maps(x, adjacency, W, attn_src, attn_dst)
    nc = _get_nc()
    res = bass_utils.run_bass_kernel_spmd(nc, in_maps, core_ids=list(range(NC)))
    # each core returns out^T [F, S]; transpose and stack shards
    return np.concatenate([r["out"].T for r in res.results], axis=0)
